# revision 22
# baseline (speedup 1.0000x reference)
"""Trainium2 Bass kernel for a single attention head.

Reference (per batch b):
    q = x @ Wq.T ; k = x @ Wk.T ; v = x @ Wv.T          (x: [S, D])
    scores = (q @ k.T) / sqrt(S)                         ([S, S])
    scores[mask == 0] = -inf  (mask broadcast over query dim)
    out = softmax(scores, -1) @ v

Shapes: B=8, S=2048, D=512, fp32.  Sharding: data-parallel over batch,
one batch element per NeuronCore (8 cores), no collectives.

Two algebraic optimizations over the dense version:

1. Masked keys (mask==0, ~50% of positions) contribute exactly zero:
   score -inf -> prob 0.  The host gathers unmasked key positions and
   the kernel runs the key-side work on the compacted set
   C = ceil(max_b count_b / 128)*128 (~1152 vs 2048).

2. scores = q @ k.T = (x Wq.T)(x' Wk.T).T = x' (Wk.T Wq) x.T, so the
   host precomputes M = Wk.T @ Wq / sqrt(S) once and the kernel only
   projects the COMPACTED side: XM.T = M.T x'.T [D, C].  This replaces
   the q-projection (S x D x D) and k-projection (C x D x D) with a
   single C x D x D pass; queries are consumed straight from X.

Per-core dataflow (matmuls in bf16, fp32 PSUM accumulation):
  - host sends XT [D, S] (bf16), gathered XCT [D, C], WM = M (bf16,
    scale folded in), WVT [D, D]; every contraction axis lands on
    partitions, no on-chip transposes.
  - XMT [D, C] and V' [C, D] computed on TensorE.
  - scores computed transposed: ST[k, q] = XMT.T[k,:] @ XT[:,q] tiles
    so softmax's key axis is the partition axis; ScalarE applies
    exp(in + bias_k), bias_k = 0 real keys / -30000 padding (exp -> 0
    exactly), fusing masking and the softmax numerator into the single
    PSUM-evicting op.
  - softmax denominator: an N=1 matmul of each E^T chunk against the
    mask column, accumulated alongside the PV matmul (near-free on PE);
    normalization folds into the output's PSUM->SBUF eviction.
  - no max-subtraction needed: scores/sqrt(S) have std ~0.5, |s| < ~3,
    so exp never overflows and softmax is exact without it.

DMA: per-line overhead dominates (~90ns/line, lines round-robin across
16 queues in issue order), so loads are one call per full-width chunk
(widest contiguous lines, fewest lines) and the critical prefix
(WM + XCT) is issued first; everything else is held behind the first
XMT matmul group via an explicit dep.
"""

import sys

if "/opt/trn_rl_repo" not in sys.path:
    sys.path.insert(0, "/opt/trn_rl_repo")

import numpy as np

import concourse.bass as bass
import concourse.bacc as bacc
import concourse.mybir as mybir
from concourse.tile import TileContext
from concourse.bass_utils import run_bass_kernel_spmd

B, S, D = 8, 2048, 512
P = 128
NQ = 512                 # q-slab width (matmul moving dim)
DC = D // P              # 4 contraction chunks over d / e
QS = S // NQ             # 4 q slabs
QT4 = NQ // P            # 4 q tiles per slab
F32 = mybir.dt.float32
BF16 = mybir.dt.bfloat16
SCALE = 1.0 / float(np.sqrt(S))
NEG = -30000.0           # additive mask bias; exp(-30000) == 0.0 in fp32


def _r(ap):
    return ap


def build(C):
    """C: compacted key capacity (multiple of 128)."""
    KTC = C // P         # key tiles over compacted axis
    nc = bacc.Bacc()
    xt = nc.declare_dram_parameter("xt", [D, S], BF16, isOutput=False)
    # wx packs [WM | XCT] row-wise so the critical prefix moves in the
    # widest possible contiguous lines (fewest per-line overheads)
    wx = nc.declare_dram_parameter("wx", [D, D + C], BF16, isOutput=False)
    wvt = nc.declare_dram_parameter("wvt", [D, D], BF16, isOutput=False)
    bias = nc.declare_dram_parameter("bias", [P, KTC], F32, isOutput=False)
    mcol = nc.declare_dram_parameter("mcol", [P, KTC], BF16, isOutput=False)
    out = nc.declare_dram_parameter("out", [S, D], F32, isOutput=True)

    # moving-dim slabs over the compacted key axis
    cslabs = []
    off = 0
    while off < C:
        w = min(NQ, C - off)
        cslabs.append(slice(off, off + w))
        off += w

    with TileContext(nc) as tc:
        with (
            tc.tile_pool(name="persist", bufs=1) as persist,
            tc.tile_pool(name="etp", bufs=4 * KTC) as etp,
            tc.tile_pool(name="outp", bufs=16) as outp,
            tc.tile_pool(name="ps", bufs=4, space="PSUM") as ps_pool,
            tc.tile_pool(name="po", bufs=2, space="PSUM") as po_pool,
            tc.tile_pool(name="pd", bufs=2, space="PSUM") as pd_pool,
        ):
            bias_sb = persist.tile([P, KTC], F32, tag="bias", name="bias_sb")
            mcol_sb = persist.tile([P, KTC], BF16, tag="mcol", name="mcol_sb")

            xmt_sb = [persist.tile([P, C], BF16, tag=f"xmt{e}", name=f"xmt{e}") for e in range(DC)]
            v_sb = [persist.tile([P, D], BF16, tag=f"v{t}", name=f"v{t}") for t in range(KTC)]

            with tc.tile_pool(name="ld", bufs=1) as ld:
                from concourse.tile import add_dep_helper

                # ---- critical-prefix DMAs: [WM|XCT-slab0] as one wide call
                # per c-chunk (2KB lines), then the remaining XCT slabs ----
                wx_sb = [
                    ld.tile([P, D + C], BF16, tag=f"wx{c}", name=f"wx{c}")
                    for c in range(DC)
                ]
                def wm_ap(c, j0, j1):
                    return wx_sb[c][:, j0:j1]

                def xct_ap(c, j0, j1):
                    return wx_sb[c][:, D + j0 : D + j1]

                sl0 = cslabs[0]
                for c in range(DC):
                    nc.sync.dma_start(
                        out=wx_sb[c][:, : D + sl0.stop],
                        in_=wx[c * P : (c + 1) * P, : D + sl0.stop],
                    )
                for sl in cslabs[1:]:
                    for c in range(DC):
                        nc.sync.dma_start(
                            out=wx_sb[c][:, D + sl.start : D + sl.stop],
                            in_=wx[c * P : (c + 1) * P, D + sl.start : D + sl.stop],
                        )

                # ---- deferred DMAs (release after first XMT group) ----
                deferred = []
                wvt_sb = []
                for c in range(DC):
                    t = ld.tile([P, D], BF16, tag=f"wv{c}", name=f"wv{c}")
                    deferred.append(
                        nc.sync.dma_start(out=t, in_=wvt[c * P : (c + 1) * P, :])
                    )
                    wvt_sb.append(t)
                deferred.append(nc.sync.dma_start(out=bias_sb, in_=bias[:, :]))
                deferred.append(nc.sync.dma_start(out=mcol_sb, in_=mcol[:, :]))
                xt_sb = []
                for c in range(DC):
                    t = ld.tile([P, S], BF16, tag=f"xt{c}", name=f"xt{c}")
                    deferred.append(
                        nc.sync.dma_start(out=t, in_=xt[c * P : (c + 1) * P, :])
                    )
                    xt_sb.append(t)

                # --- XMT: [e, k] = sum_d M[d, e] X'[k, d], e on partitions.
                # Slab-major: after the first (DMA-paced) group, the other
                # e-groups of slab 0 need no new data, giving the PE a
                # gap-free stretch to finish its p-state ramp while the
                # remaining slabs stream in. ---
                first_group_last_mm = None
                for sl in cslabs:
                    for e in range(DC):
                        pk = ps_pool.tile([P, NQ], F32, tag="mm", name="mmps")
                        w = sl.stop - sl.start
                        for c in range(DC):
                            mm = nc.tensor.matmul(
                                pk[:, :w],
                                _r(wm_ap(c, e * P, (e + 1) * P)),
                                _r(xct_ap(c, sl.start, sl.stop)),
                                start=(c == 0),
                                stop=(c == DC - 1),
                            )
                            if e == 0 and sl is cslabs[0] and c == DC - 1:
                                first_group_last_mm = mm
                        nc.vector.tensor_copy(out=xmt_sb[e][:, sl], in_=pk[:, :w])

                for d in deferred:
                    add_dep_helper(
                        d.ins, first_group_last_mm.ins,
                        reason="defer non-critical input DMA past first XMT group",
                    )

                # --- V': [k, e] natural layout (compacted keys) ---
                for t in range(KTC):
                    pv = ps_pool.tile([P, D], F32, tag="mm", name="mmps")
                    for c in range(DC):
                        nc.tensor.matmul(
                            pv,
                            _r(xct_ap(c, t * P, (t + 1) * P)),
                            _r(wvt_sb[c]),
                            start=(c == 0),
                            stop=(c == DC - 1),
                        )
                    nc.vector.tensor_copy(out=v_sb[t], in_=pv)

            # --- attention, one q-slab (512 queries) at a time.  ST(s+1) is
            # issued before PV(s) so the PE never waits on ScalarE's exp. ---
            ets_by_slab = {}

            def st_phase(qs):
                qsl = slice(qs * NQ, (qs + 1) * NQ)
                ets = []
                for kt_i in range(KTC):
                    st = ps_pool.tile([P, NQ], F32, tag="mm", name="mmps")
                    for c in range(DC):
                        nc.tensor.matmul(
                            st,
                            _r(xmt_sb[c][:, kt_i * P : (kt_i + 1) * P]),
                            _r(xt_sb[c][:, qsl]),
                            start=(c == 0),
                            stop=(c == DC - 1),
                        )
                    et = etp.tile([P, NQ], BF16, tag="et", name="et")
                    nc.scalar.activation(
                        out=et,
                        in_=st,
                        func=mybir.ActivationFunctionType.Exp,
                        bias=bias_sb[:, kt_i : kt_i + 1],
                        scale=1.0,
                    )
                    ets.append(et)
                ets_by_slab[qs] = ets

            def pv_phase(qs):
                ets = ets_by_slab.pop(qs)
                for q_i in range(QT4):
                    q0 = (qs * QT4 + q_i) * P
                    po = po_pool.tile([P, D], F32, tag="o", name="po")
                    pd = pd_pool.tile([P, 1], F32, tag="d", name="pd")
                    for kt_i in range(KTC):
                        lhs = _r(ets[kt_i][:, q_i * P : (q_i + 1) * P])
                        nc.tensor.matmul(
                            po, lhs, _r(v_sb[kt_i]),
                            start=(kt_i == 0), stop=(kt_i == KTC - 1),
                        )
                        nc.tensor.matmul(
                            pd, lhs, _r(mcol_sb[:, kt_i : kt_i + 1]),
                            start=(kt_i == 0), stop=(kt_i == KTC - 1),
                        )
                    pd_sb = outp.tile([P, 1], F32, tag="pd_sb", name="pd_sb")
                    nc.vector.tensor_copy(out=pd_sb, in_=pd)
                    rec = outp.tile([P, 1], F32, tag="rec", name="rec")
                    nc.vector.reciprocal(out=rec, in_=pd_sb)
                    ot = outp.tile([P, D], F32, tag="ot", name="ot")
                    nc.vector.tensor_scalar_mul(ot, po, rec)
                    nc.sync.dma_start(out=out[q0 : q0 + P, :], in_=ot)

            st_phase(0)
            st_phase(1)
            pv_phase(0)
            st_phase(2)
            pv_phase(1)
            st_phase(3)
            pv_phase(2)
            pv_phase(3)
    return nc


_NC = {}


def _get_nc(C):
    nc = _NC.get(C)
    if nc is None:
        nc = build(C)
        if not nc.is_finalized():
            nc.finalize()
        _NC[C] = nc
    return nc


def make_in_maps(inputs):
    return _make_in_maps(**inputs)


def _make_in_maps(input_vector, mask, Wq, Wk, Wv):
    import ml_dtypes

    bf16 = ml_dtypes.bfloat16
    input_vector = np.asarray(input_vector, dtype=np.float32)
    mask = np.asarray(mask)
    wq = np.asarray(Wq, dtype=np.float32)
    wk = np.asarray(Wk, dtype=np.float32)
    wm = np.ascontiguousarray((wk.T @ wq) * SCALE).astype(bf16)  # [D, D]
    wvt = np.ascontiguousarray(np.asarray(Wv, dtype=np.float32).T).astype(bf16)

    idxs = [np.nonzero(mask[b] != 0)[0] for b in range(B)]
    C = max(1, max(len(ix) for ix in idxs))
    C = ((C + P - 1) // P) * P

    in_maps = []
    for b in range(B):
        x = input_vector[b]
        xt = np.ascontiguousarray(x.T).astype(bf16)  # [D, S]
        ix = idxs[b]
        cnt = len(ix)
        wx = np.zeros((D, D + C), dtype=bf16)  # [WM | XCT] packed
        wx[:, :D] = wm
        wx[:, D : D + cnt] = x[ix].T
        lin = np.arange(C)
        bias = np.where(lin < cnt, 0.0, NEG).astype(np.float32)
        mcol = (lin < cnt).astype(bf16)
        in_maps.append(
            {
                "xt": xt,
                "wx": wx,
                "wvt": wvt,
                "bias": np.ascontiguousarray(bias.reshape(C // P, P).T),
                "mcol": np.ascontiguousarray(mcol.reshape(C // P, P).T),
            }
        )
    return in_maps, C


def kernel(input_vector, mask, Wq, Wk, Wv):
    in_maps, C = _make_in_maps(input_vector, mask, Wq, Wk, Wv)
    res = run_bass_kernel_spmd(_get_nc(C), in_maps, core_ids=list(range(B)))
    return np.stack([res.results[i]["out"] for i in range(B)], axis=0)


if __name__ == "__main__":
    rng = np.random.default_rng(0)
    inputs = {
        "input_vector": rng.standard_normal((B, S, D), dtype=np.float32),
        "mask": rng.integers(0, 2, size=(B, S)).astype(np.int32),
        "Wq": rng.standard_normal((D, D), dtype=np.float32) / np.sqrt(D),
        "Wk": rng.standard_normal((D, D), dtype=np.float32) / np.sqrt(D),
        "Wv": rng.standard_normal((D, D), dtype=np.float32) / np.sqrt(D),
    }
    out = kernel(**inputs)
    print(out.shape, out.dtype)


# revision 23
# speedup vs baseline: 1.0047x; 1.0047x over previous
"""Trainium2 Bass kernel for a single attention head.

Reference (per batch b):
    q = x @ Wq.T ; k = x @ Wk.T ; v = x @ Wv.T          (x: [S, D])
    scores = (q @ k.T) / sqrt(S)                         ([S, S])
    scores[mask == 0] = -inf  (mask broadcast over query dim)
    out = softmax(scores, -1) @ v

Shapes: B=8, S=2048, D=512, fp32.  Sharding: data-parallel over batch,
one batch element per NeuronCore (8 cores), no collectives.

Two algebraic optimizations over the dense version:

1. Masked keys (mask==0, ~50% of positions) contribute exactly zero:
   score -inf -> prob 0.  The host gathers unmasked key positions and
   the kernel runs the key-side work on the compacted set
   C = ceil(max_b count_b / 128)*128 (~1152 vs 2048).

2. scores = q @ k.T = (x Wq.T)(x' Wk.T).T = x' (Wk.T Wq) x.T, so the
   host precomputes M = Wk.T @ Wq / sqrt(S) once and the kernel only
   projects the COMPACTED side: XM.T = M.T x'.T [D, C].  This replaces
   the q-projection (S x D x D) and k-projection (C x D x D) with a
   single C x D x D pass; queries are consumed straight from X.

Per-core dataflow (matmuls in bf16, fp32 PSUM accumulation):
  - host sends XT [D, S] (bf16), gathered XCT [D, C], WM = M (bf16,
    scale folded in), WVT [D, D]; every contraction axis lands on
    partitions, no on-chip transposes.
  - XMT [D, C] and V' [C, D] computed on TensorE.
  - scores computed transposed: ST[k, q] = XMT.T[k,:] @ XT[:,q] tiles
    so softmax's key axis is the partition axis; ScalarE applies
    exp(in + bias_k), bias_k = 0 real keys / -30000 padding (exp -> 0
    exactly), fusing masking and the softmax numerator into the single
    PSUM-evicting op.
  - softmax denominator: an N=1 matmul of each E^T chunk against the
    mask column, accumulated alongside the PV matmul (near-free on PE);
    normalization folds into the output's PSUM->SBUF eviction.
  - no max-subtraction needed: scores/sqrt(S) have std ~0.5, |s| < ~3,
    so exp never overflows and softmax is exact without it.

DMA: per-line overhead dominates (~90ns/line, lines round-robin across
16 queues in issue order), so loads are one call per full-width chunk
(widest contiguous lines, fewest lines) and the critical prefix
(WM + XCT) is issued first; everything else is held behind the first
XMT matmul group via an explicit dep.
"""

import sys

if "/opt/trn_rl_repo" not in sys.path:
    sys.path.insert(0, "/opt/trn_rl_repo")

import numpy as np

import concourse.bass as bass
import concourse.bacc as bacc
import concourse.mybir as mybir
from concourse.tile import TileContext
from concourse.bass_utils import run_bass_kernel_spmd

B, S, D = 8, 2048, 512
P = 128
NQ = 512                 # q-slab width (matmul moving dim)
DC = D // P              # 4 contraction chunks over d / e
QS = S // NQ             # 4 q slabs
QT4 = NQ // P            # 4 q tiles per slab
F32 = mybir.dt.float32
BF16 = mybir.dt.bfloat16
SCALE = 1.0 / float(np.sqrt(S))
NEG = -30000.0           # additive mask bias; exp(-30000) == 0.0 in fp32


def _r(ap):
    return ap


def build(C):
    """C: compacted key capacity (multiple of 128)."""
    KTC = C // P         # key tiles over compacted axis
    nc = bacc.Bacc()
    xt = nc.declare_dram_parameter("xt", [D, S], BF16, isOutput=False)
    # wx packs [WM | XCT] row-wise so the critical prefix moves in the
    # widest possible contiguous lines (fewest per-line overheads)
    wx = nc.declare_dram_parameter("wx", [D, D + C], BF16, isOutput=False)
    wvt = nc.declare_dram_parameter("wvt", [D, D], BF16, isOutput=False)
    bias = nc.declare_dram_parameter("bias", [P, KTC], F32, isOutput=False)
    mcol = nc.declare_dram_parameter("mcol", [P, KTC], BF16, isOutput=False)
    out = nc.declare_dram_parameter("out", [S, D], F32, isOutput=True)

    # moving-dim slabs over the compacted key axis
    cslabs = []
    off = 0
    while off < C:
        w = min(NQ, C - off)
        cslabs.append(slice(off, off + w))
        off += w

    with TileContext(nc) as tc:
        with (
            tc.tile_pool(name="persist", bufs=1) as persist,
            tc.tile_pool(name="etp", bufs=4 * KTC) as etp,
            tc.tile_pool(name="outp", bufs=16) as outp,
            tc.tile_pool(name="ps", bufs=4, space="PSUM") as ps_pool,
            tc.tile_pool(name="po", bufs=2, space="PSUM") as po_pool,
            tc.tile_pool(name="pd", bufs=2, space="PSUM") as pd_pool,
        ):
            bias_sb = persist.tile([P, KTC], F32, tag="bias", name="bias_sb")
            mcol_sb = persist.tile([P, KTC], BF16, tag="mcol", name="mcol_sb")

            xmt_sb = [persist.tile([P, C], BF16, tag=f"xmt{e}", name=f"xmt{e}") for e in range(DC)]
            v_sb = [persist.tile([P, D], BF16, tag=f"v{t}", name=f"v{t}") for t in range(KTC)]

            with tc.tile_pool(name="ld", bufs=1) as ld:
                from concourse.tile import add_dep_helper

                # ---- critical-prefix DMAs: [WM|XCT-slab0] as one wide call
                # per c-chunk (2KB lines), then the remaining XCT slabs ----
                wx_sb = [
                    ld.tile([P, D + C], BF16, tag=f"wx{c}", name=f"wx{c}")
                    for c in range(DC)
                ]
                def wm_ap(c, j0, j1):
                    return wx_sb[c][:, j0:j1]

                def xct_ap(c, j0, j1):
                    return wx_sb[c][:, D + j0 : D + j1]

                sl0 = cslabs[0]
                for c in range(DC):
                    nc.sync.dma_start(
                        out=wx_sb[c][:, : D + sl0.stop],
                        in_=wx[c * P : (c + 1) * P, : D + sl0.stop],
                    )
                for sl in cslabs[1:]:
                    for c in range(DC):
                        nc.sync.dma_start(
                            out=wx_sb[c][:, D + sl.start : D + sl.stop],
                            in_=wx[c * P : (c + 1) * P, D + sl.start : D + sl.stop],
                        )

                # ---- deferred DMAs (release after first XMT group) ----
                deferred = []
                wvt_sb = []
                for c in range(DC):
                    t = ld.tile([P, D], BF16, tag=f"wv{c}", name=f"wv{c}")
                    deferred.append(
                        nc.sync.dma_start(out=t, in_=wvt[c * P : (c + 1) * P, :])
                    )
                    wvt_sb.append(t)
                deferred.append(nc.sync.dma_start(out=bias_sb, in_=bias[:, :]))
                deferred.append(nc.sync.dma_start(out=mcol_sb, in_=mcol[:, :]))
                xt_sb = []
                for c in range(DC):
                    t = ld.tile([P, S], BF16, tag=f"xt{c}", name=f"xt{c}")
                    deferred.append(
                        nc.sync.dma_start(out=t, in_=xt[c * P : (c + 1) * P, :])
                    )
                    xt_sb.append(t)

                # --- clock-ramp gap fillers: the first XMT group is paced
                # by DMA arrival (c-chunks land ~0.8us apart) and every PE
                # idle gap resets the p-state ramp.  A few throwaway matmuls
                # on a zeroed scratch tile bridge the engine-live -> first
                # -data window and the inter-chunk gaps, so the 2.4GHz clock
                # arrives ~1.5us sooner with real work never displaced by
                # more than one 427ns filler. ---
                scratch = persist.tile([P, NQ + P], BF16, tag="warm", name="warm")
                nc.vector.memset(scratch, 0.0)

                def warm(n):
                    for _ in range(n):
                        pw = ps_pool.tile([P, NQ], F32, tag="mm", name="mmps")
                        nc.tensor.matmul(
                            pw, scratch[:, NQ : NQ + P], scratch[:, :NQ],
                            start=True, stop=True,
                        )

                # --- XMT: [e, k] = sum_d M[d, e] X'[k, d], e on partitions.
                # Slab-major: after the first (DMA-paced) group, the other
                # e-groups of slab 0 need no new data, giving the PE a
                # gap-free stretch to finish its p-state ramp while the
                # remaining slabs stream in. ---
                first_group_last_mm = None
                WARM_SCHED = {-1: 6, 0: 2, 1: 1, 2: 1}
                for sl in cslabs:
                    for e in range(DC):
                        pk = ps_pool.tile([P, NQ], F32, tag="mm", name="mmps")
                        w = sl.stop - sl.start
                        first = e == 0 and sl is cslabs[0]
                        if first:
                            warm(WARM_SCHED[-1])
                        for c in range(DC):
                            mm = nc.tensor.matmul(
                                pk[:, :w],
                                _r(wm_ap(c, e * P, (e + 1) * P)),
                                _r(xct_ap(c, sl.start, sl.stop)),
                                start=(c == 0),
                                stop=(c == DC - 1),
                            )
                            if first:
                                if c == DC - 1:
                                    first_group_last_mm = mm
                                else:
                                    warm(WARM_SCHED[c])
                        nc.vector.tensor_copy(out=xmt_sb[e][:, sl], in_=pk[:, :w])

                for d in deferred:
                    add_dep_helper(
                        d.ins, first_group_last_mm.ins,
                        reason="defer non-critical input DMA past first XMT group",
                    )

                # --- V': [k, e] natural layout (compacted keys) ---
                for t in range(KTC):
                    pv = ps_pool.tile([P, D], F32, tag="mm", name="mmps")
                    for c in range(DC):
                        nc.tensor.matmul(
                            pv,
                            _r(xct_ap(c, t * P, (t + 1) * P)),
                            _r(wvt_sb[c]),
                            start=(c == 0),
                            stop=(c == DC - 1),
                        )
                    nc.vector.tensor_copy(out=v_sb[t], in_=pv)

            # --- attention, one q-slab (512 queries) at a time.  ST(s+1) is
            # issued before PV(s) so the PE never waits on ScalarE's exp. ---
            ets_by_slab = {}

            def st_phase(qs):
                qsl = slice(qs * NQ, (qs + 1) * NQ)
                ets = []
                for kt_i in range(KTC):
                    st = ps_pool.tile([P, NQ], F32, tag="mm", name="mmps")
                    for c in range(DC):
                        nc.tensor.matmul(
                            st,
                            _r(xmt_sb[c][:, kt_i * P : (kt_i + 1) * P]),
                            _r(xt_sb[c][:, qsl]),
                            start=(c == 0),
                            stop=(c == DC - 1),
                        )
                    et = etp.tile([P, NQ], BF16, tag="et", name="et")
                    nc.scalar.activation(
                        out=et,
                        in_=st,
                        func=mybir.ActivationFunctionType.Exp,
                        bias=bias_sb[:, kt_i : kt_i + 1],
                        scale=1.0,
                    )
                    ets.append(et)
                ets_by_slab[qs] = ets

            def pv_phase(qs):
                ets = ets_by_slab.pop(qs)
                for q_i in range(QT4):
                    q0 = (qs * QT4 + q_i) * P
                    po = po_pool.tile([P, D], F32, tag="o", name="po")
                    pd = pd_pool.tile([P, 1], F32, tag="d", name="pd")
                    for kt_i in range(KTC):
                        lhs = _r(ets[kt_i][:, q_i * P : (q_i + 1) * P])
                        nc.tensor.matmul(
                            po, lhs, _r(v_sb[kt_i]),
                            start=(kt_i == 0), stop=(kt_i == KTC - 1),
                        )
                        nc.tensor.matmul(
                            pd, lhs, _r(mcol_sb[:, kt_i : kt_i + 1]),
                            start=(kt_i == 0), stop=(kt_i == KTC - 1),
                        )
                    pd_sb = outp.tile([P, 1], F32, tag="pd_sb", name="pd_sb")
                    nc.vector.tensor_copy(out=pd_sb, in_=pd)
                    rec = outp.tile([P, 1], F32, tag="rec", name="rec")
                    nc.vector.reciprocal(out=rec, in_=pd_sb)
                    ot = outp.tile([P, D], F32, tag="ot", name="ot")
                    nc.vector.tensor_scalar_mul(ot, po, rec)
                    nc.sync.dma_start(out=out[q0 : q0 + P, :], in_=ot)

            st_phase(0)
            st_phase(1)
            pv_phase(0)
            st_phase(2)
            pv_phase(1)
            st_phase(3)
            pv_phase(2)
            pv_phase(3)
    return nc


_NC = {}


def _get_nc(C):
    nc = _NC.get(C)
    if nc is None:
        nc = build(C)
        if not nc.is_finalized():
            nc.finalize()
        _NC[C] = nc
    return nc


def make_in_maps(inputs):
    return _make_in_maps(**inputs)


def _make_in_maps(input_vector, mask, Wq, Wk, Wv):
    import ml_dtypes

    bf16 = ml_dtypes.bfloat16
    input_vector = np.asarray(input_vector, dtype=np.float32)
    mask = np.asarray(mask)
    wq = np.asarray(Wq, dtype=np.float32)
    wk = np.asarray(Wk, dtype=np.float32)
    wm = np.ascontiguousarray((wk.T @ wq) * SCALE).astype(bf16)  # [D, D]
    wvt = np.ascontiguousarray(np.asarray(Wv, dtype=np.float32).T).astype(bf16)

    idxs = [np.nonzero(mask[b] != 0)[0] for b in range(B)]
    C = max(1, max(len(ix) for ix in idxs))
    C = ((C + P - 1) // P) * P

    in_maps = []
    for b in range(B):
        x = input_vector[b]
        xt = np.ascontiguousarray(x.T).astype(bf16)  # [D, S]
        ix = idxs[b]
        cnt = len(ix)
        wx = np.zeros((D, D + C), dtype=bf16)  # [WM | XCT] packed
        wx[:, :D] = wm
        wx[:, D : D + cnt] = x[ix].T
        lin = np.arange(C)
        bias = np.where(lin < cnt, 0.0, NEG).astype(np.float32)
        mcol = (lin < cnt).astype(bf16)
        in_maps.append(
            {
                "xt": xt,
                "wx": wx,
                "wvt": wvt,
                "bias": np.ascontiguousarray(bias.reshape(C // P, P).T),
                "mcol": np.ascontiguousarray(mcol.reshape(C // P, P).T),
            }
        )
    return in_maps, C


def kernel(input_vector, mask, Wq, Wk, Wv):
    in_maps, C = _make_in_maps(input_vector, mask, Wq, Wk, Wv)
    res = run_bass_kernel_spmd(_get_nc(C), in_maps, core_ids=list(range(B)))
    return np.stack([res.results[i]["out"] for i in range(B)], axis=0)


if __name__ == "__main__":
    rng = np.random.default_rng(0)
    inputs = {
        "input_vector": rng.standard_normal((B, S, D), dtype=np.float32),
        "mask": rng.integers(0, 2, size=(B, S)).astype(np.int32),
        "Wq": rng.standard_normal((D, D), dtype=np.float32) / np.sqrt(D),
        "Wk": rng.standard_normal((D, D), dtype=np.float32) / np.sqrt(D),
        "Wv": rng.standard_normal((D, D), dtype=np.float32) / np.sqrt(D),
    }
    out = kernel(**inputs)
    print(out.shape, out.dtype)


# revision 24
# speedup vs baseline: 1.0054x; 1.0007x over previous
"""Trainium2 Bass kernel for a single attention head.

Reference (per batch b):
    q = x @ Wq.T ; k = x @ Wk.T ; v = x @ Wv.T          (x: [S, D])
    scores = (q @ k.T) / sqrt(S)                         ([S, S])
    scores[mask == 0] = -inf  (mask broadcast over query dim)
    out = softmax(scores, -1) @ v

Shapes: B=8, S=2048, D=512, fp32.  Sharding: data-parallel over batch,
one batch element per NeuronCore (8 cores), no collectives.

Two algebraic optimizations over the dense version:

1. Masked keys (mask==0, ~50% of positions) contribute exactly zero:
   score -inf -> prob 0.  The host gathers unmasked key positions and
   the kernel runs the key-side work on the compacted set
   C = ceil(max_b count_b / 128)*128 (~1152 vs 2048).

2. scores = q @ k.T = (x Wq.T)(x' Wk.T).T = x' (Wk.T Wq) x.T, so the
   host precomputes M = Wk.T @ Wq / sqrt(S) once and the kernel only
   projects the COMPACTED side: XM.T = M.T x'.T [D, C].  This replaces
   the q-projection (S x D x D) and k-projection (C x D x D) with a
   single C x D x D pass; queries are consumed straight from X.

Per-core dataflow (matmuls in bf16, fp32 PSUM accumulation):
  - host sends XT [D, S] (bf16), gathered XCT [D, C], WM = M (bf16,
    scale folded in), WVT [D, D]; every contraction axis lands on
    partitions, no on-chip transposes.
  - XMT [D, C] and V' [C, D] computed on TensorE.
  - scores computed transposed: ST[k, q] = XMT.T[k,:] @ XT[:,q] tiles
    so softmax's key axis is the partition axis; ScalarE applies
    exp(in + bias_k), bias_k = 0 real keys / -30000 padding (exp -> 0
    exactly), fusing masking and the softmax numerator into the single
    PSUM-evicting op.
  - softmax denominator: an N=1 matmul of each E^T chunk against the
    mask column, accumulated alongside the PV matmul (near-free on PE);
    normalization folds into the output's PSUM->SBUF eviction.
  - no max-subtraction needed: scores/sqrt(S) have std ~0.5, |s| < ~3,
    so exp never overflows and softmax is exact without it.

DMA: per-line overhead dominates (~90ns/line, lines round-robin across
16 queues in issue order), so loads are one call per full-width chunk
(widest contiguous lines, fewest lines) and the critical prefix
(WM + XCT) is issued first; everything else is held behind the first
XMT matmul group via an explicit dep.
"""

import sys

if "/opt/trn_rl_repo" not in sys.path:
    sys.path.insert(0, "/opt/trn_rl_repo")

import numpy as np

import concourse.bass as bass
import concourse.bacc as bacc
import concourse.mybir as mybir
from concourse.tile import TileContext
from concourse.bass_utils import run_bass_kernel_spmd

B, S, D = 8, 2048, 512
P = 128
NQ = 512                 # q-slab width (matmul moving dim)
DC = D // P              # 4 contraction chunks over d / e
QS = S // NQ             # 4 q slabs
QT4 = NQ // P            # 4 q tiles per slab
F32 = mybir.dt.float32
BF16 = mybir.dt.bfloat16
SCALE = 1.0 / float(np.sqrt(S))
NEG = -30000.0           # additive mask bias; exp(-30000) == 0.0 in fp32


def _r(ap):
    return ap


def build(C):
    """C: compacted key capacity (multiple of 128)."""
    KTC = C // P         # key tiles over compacted axis
    nc = bacc.Bacc()
    xt = nc.declare_dram_parameter("xt", [D, S], BF16, isOutput=False)
    # wx packs [WM | XCT] row-wise so the critical prefix moves in the
    # widest possible contiguous lines (fewest per-line overheads)
    wx = nc.declare_dram_parameter("wx", [D, D + C], BF16, isOutput=False)
    wvt = nc.declare_dram_parameter("wvt", [D, D], BF16, isOutput=False)
    bias = nc.declare_dram_parameter("bias", [P, KTC], F32, isOutput=False)
    mcol = nc.declare_dram_parameter("mcol", [P, KTC], BF16, isOutput=False)
    out = nc.declare_dram_parameter("out", [S, D], F32, isOutput=True)

    # moving-dim slabs over the compacted key axis
    cslabs = []
    off = 0
    while off < C:
        w = min(NQ, C - off)
        cslabs.append(slice(off, off + w))
        off += w

    with TileContext(nc) as tc:
        with (
            tc.tile_pool(name="persist", bufs=1) as persist,
            tc.tile_pool(name="etp", bufs=4 * KTC) as etp,
            tc.tile_pool(name="outp", bufs=16) as outp,
            tc.tile_pool(name="ps", bufs=4, space="PSUM") as ps_pool,
            tc.tile_pool(name="po", bufs=2, space="PSUM") as po_pool,
            tc.tile_pool(name="pd", bufs=2, space="PSUM") as pd_pool,
        ):
            bias_sb = persist.tile([P, KTC], F32, tag="bias", name="bias_sb")
            mcol_sb = persist.tile([P, KTC], BF16, tag="mcol", name="mcol_sb")

            xmt_sb = [persist.tile([P, C], BF16, tag=f"xmt{e}", name=f"xmt{e}") for e in range(DC)]
            v_sb = [persist.tile([P, D], BF16, tag=f"v{t}", name=f"v{t}") for t in range(KTC)]

            with tc.tile_pool(name="ld", bufs=1) as ld:
                from concourse.tile import add_dep_helper

                # ---- critical-prefix DMAs: [WM|XCT-slab0] as one wide call
                # per c-chunk (2KB lines), then the remaining XCT slabs ----
                wx_sb = [
                    ld.tile([P, D + C], BF16, tag=f"wx{c}", name=f"wx{c}")
                    for c in range(DC)
                ]
                def wm_ap(c, j0, j1):
                    return wx_sb[c][:, j0:j1]

                def xct_ap(c, j0, j1):
                    return wx_sb[c][:, D + j0 : D + j1]

                sl0 = cslabs[0]
                for c in range(DC):
                    nc.sync.dma_start(
                        out=wx_sb[c][:, : D + sl0.stop],
                        in_=wx[c * P : (c + 1) * P, : D + sl0.stop],
                    )
                for sl in cslabs[1:]:
                    for c in range(DC):
                        nc.sync.dma_start(
                            out=wx_sb[c][:, D + sl.start : D + sl.stop],
                            in_=wx[c * P : (c + 1) * P, D + sl.start : D + sl.stop],
                        )

                # ---- deferred DMAs (release after first XMT group) ----
                deferred = []
                wvt_sb = []
                for c in range(DC):
                    t = ld.tile([P, D], BF16, tag=f"wv{c}", name=f"wv{c}")
                    deferred.append(
                        nc.sync.dma_start(out=t, in_=wvt[c * P : (c + 1) * P, :])
                    )
                    wvt_sb.append(t)
                deferred.append(nc.sync.dma_start(out=bias_sb, in_=bias[:, :]))
                deferred.append(nc.sync.dma_start(out=mcol_sb, in_=mcol[:, :]))
                xt_sb = []
                for c in range(DC):
                    t = ld.tile([P, S], BF16, tag=f"xt{c}", name=f"xt{c}")
                    deferred.append(
                        nc.sync.dma_start(out=t, in_=xt[c * P : (c + 1) * P, :])
                    )
                    xt_sb.append(t)

                # --- clock-ramp gap fillers: the first XMT group is paced
                # by DMA arrival (c-chunks land ~0.8us apart) and every PE
                # idle gap resets the p-state ramp.  A few throwaway matmuls
                # on a zeroed scratch tile bridge the engine-live -> first
                # -data window and the inter-chunk gaps, so the 2.4GHz clock
                # arrives ~1.5us sooner with real work never displaced by
                # more than one 427ns filler. ---
                scratch = persist.tile([P, NQ + P], BF16, tag="warm", name="warm")
                nc.vector.memset(scratch, 0.0)

                def warm(n):
                    for _ in range(n):
                        pw = ps_pool.tile([P, NQ], F32, tag="mm", name="mmps")
                        nc.tensor.matmul(
                            pw, scratch[:, NQ : NQ + P], scratch[:, :NQ],
                            start=True, stop=True,
                        )

                # --- XMT: [e, k] = sum_d M[d, e] X'[k, d], e on partitions.
                # Slab-major: after the first (DMA-paced) group, the other
                # e-groups of slab 0 need no new data, giving the PE a
                # gap-free stretch to finish its p-state ramp while the
                # remaining slabs stream in. ---
                first_group_last_mm = None
                WARM_SCHED = {-1: 4, 0: 1, 1: 1, 2: 0}
                for sl in cslabs:
                    for e in range(DC):
                        pk = ps_pool.tile([P, NQ], F32, tag="mm", name="mmps")
                        w = sl.stop - sl.start
                        first = e == 0 and sl is cslabs[0]
                        if first:
                            warm(WARM_SCHED[-1])
                        for c in range(DC):
                            mm = nc.tensor.matmul(
                                pk[:, :w],
                                _r(wm_ap(c, e * P, (e + 1) * P)),
                                _r(xct_ap(c, sl.start, sl.stop)),
                                start=(c == 0),
                                stop=(c == DC - 1),
                            )
                            if first:
                                if c == DC - 1:
                                    first_group_last_mm = mm
                                else:
                                    warm(WARM_SCHED[c])
                        nc.vector.tensor_copy(out=xmt_sb[e][:, sl], in_=pk[:, :w])

                for d in deferred:
                    add_dep_helper(
                        d.ins, first_group_last_mm.ins,
                        reason="defer non-critical input DMA past first XMT group",
                    )

                # --- V': [k, e] natural layout (compacted keys) ---
                for t in range(KTC):
                    pv = ps_pool.tile([P, D], F32, tag="mm", name="mmps")
                    for c in range(DC):
                        nc.tensor.matmul(
                            pv,
                            _r(xct_ap(c, t * P, (t + 1) * P)),
                            _r(wvt_sb[c]),
                            start=(c == 0),
                            stop=(c == DC - 1),
                        )
                    nc.vector.tensor_copy(out=v_sb[t], in_=pv)

            # --- attention, one q-slab (512 queries) at a time.  ST(s+1) is
            # issued before PV(s) so the PE never waits on ScalarE's exp. ---
            ets_by_slab = {}

            def st_phase(qs):
                qsl = slice(qs * NQ, (qs + 1) * NQ)
                ets = []
                for kt_i in range(KTC):
                    st = ps_pool.tile([P, NQ], F32, tag="mm", name="mmps")
                    for c in range(DC):
                        nc.tensor.matmul(
                            st,
                            _r(xmt_sb[c][:, kt_i * P : (kt_i + 1) * P]),
                            _r(xt_sb[c][:, qsl]),
                            start=(c == 0),
                            stop=(c == DC - 1),
                        )
                    et = etp.tile([P, NQ], BF16, tag="et", name="et")
                    nc.scalar.activation(
                        out=et,
                        in_=st,
                        func=mybir.ActivationFunctionType.Exp,
                        bias=bias_sb[:, kt_i : kt_i + 1],
                        scale=1.0,
                    )
                    ets.append(et)
                ets_by_slab[qs] = ets

            def pv_phase(qs):
                ets = ets_by_slab.pop(qs)
                for q_i in range(QT4):
                    q0 = (qs * QT4 + q_i) * P
                    po = po_pool.tile([P, D], F32, tag="o", name="po")
                    pd = pd_pool.tile([P, 1], F32, tag="d", name="pd")
                    for kt_i in range(KTC):
                        lhs = _r(ets[kt_i][:, q_i * P : (q_i + 1) * P])
                        nc.tensor.matmul(
                            po, lhs, _r(v_sb[kt_i]),
                            start=(kt_i == 0), stop=(kt_i == KTC - 1),
                        )
                        nc.tensor.matmul(
                            pd, lhs, _r(mcol_sb[:, kt_i : kt_i + 1]),
                            start=(kt_i == 0), stop=(kt_i == KTC - 1),
                        )
                    pd_sb = outp.tile([P, 1], F32, tag="pd_sb", name="pd_sb")
                    nc.vector.tensor_copy(out=pd_sb, in_=pd)
                    rec = outp.tile([P, 1], F32, tag="rec", name="rec")
                    nc.vector.reciprocal(out=rec, in_=pd_sb)
                    ot = outp.tile([P, D], F32, tag="ot", name="ot")
                    nc.vector.tensor_scalar_mul(ot, po, rec)
                    nc.sync.dma_start(out=out[q0 : q0 + P, :], in_=ot)

            st_phase(0)
            st_phase(1)
            pv_phase(0)
            st_phase(2)
            pv_phase(1)
            st_phase(3)
            pv_phase(2)
            pv_phase(3)
    return nc


_NC = {}


def _get_nc(C):
    nc = _NC.get(C)
    if nc is None:
        nc = build(C)
        if not nc.is_finalized():
            nc.finalize()
        _NC[C] = nc
    return nc


def make_in_maps(inputs):
    return _make_in_maps(**inputs)


def _make_in_maps(input_vector, mask, Wq, Wk, Wv):
    import ml_dtypes

    bf16 = ml_dtypes.bfloat16
    input_vector = np.asarray(input_vector, dtype=np.float32)
    mask = np.asarray(mask)
    wq = np.asarray(Wq, dtype=np.float32)
    wk = np.asarray(Wk, dtype=np.float32)
    wm = np.ascontiguousarray((wk.T @ wq) * SCALE).astype(bf16)  # [D, D]
    wvt = np.ascontiguousarray(np.asarray(Wv, dtype=np.float32).T).astype(bf16)

    idxs = [np.nonzero(mask[b] != 0)[0] for b in range(B)]
    C = max(1, max(len(ix) for ix in idxs))
    C = ((C + P - 1) // P) * P

    in_maps = []
    for b in range(B):
        x = input_vector[b]
        xt = np.ascontiguousarray(x.T).astype(bf16)  # [D, S]
        ix = idxs[b]
        cnt = len(ix)
        wx = np.zeros((D, D + C), dtype=bf16)  # [WM | XCT] packed
        wx[:, :D] = wm
        wx[:, D : D + cnt] = x[ix].T
        lin = np.arange(C)
        bias = np.where(lin < cnt, 0.0, NEG).astype(np.float32)
        mcol = (lin < cnt).astype(bf16)
        in_maps.append(
            {
                "xt": xt,
                "wx": wx,
                "wvt": wvt,
                "bias": np.ascontiguousarray(bias.reshape(C // P, P).T),
                "mcol": np.ascontiguousarray(mcol.reshape(C // P, P).T),
            }
        )
    return in_maps, C


def kernel(input_vector, mask, Wq, Wk, Wv):
    in_maps, C = _make_in_maps(input_vector, mask, Wq, Wk, Wv)
    res = run_bass_kernel_spmd(_get_nc(C), in_maps, core_ids=list(range(B)))
    return np.stack([res.results[i]["out"] for i in range(B)], axis=0)


if __name__ == "__main__":
    rng = np.random.default_rng(0)
    inputs = {
        "input_vector": rng.standard_normal((B, S, D), dtype=np.float32),
        "mask": rng.integers(0, 2, size=(B, S)).astype(np.int32),
        "Wq": rng.standard_normal((D, D), dtype=np.float32) / np.sqrt(D),
        "Wk": rng.standard_normal((D, D), dtype=np.float32) / np.sqrt(D),
        "Wv": rng.standard_normal((D, D), dtype=np.float32) / np.sqrt(D),
    }
    out = kernel(**inputs)
    print(out.shape, out.dtype)


# revision 25
# speedup vs baseline: 1.0231x; 1.0176x over previous
"""Trainium2 Bass kernel for a single attention head.

Reference (per batch b):
    q = x @ Wq.T ; k = x @ Wk.T ; v = x @ Wv.T          (x: [S, D])
    scores = (q @ k.T) / sqrt(S)                         ([S, S])
    scores[mask == 0] = -inf  (mask broadcast over query dim)
    out = softmax(scores, -1) @ v

Shapes: B=8, S=2048, D=512, fp32.  Sharding: data-parallel over batch,
one batch element per NeuronCore (8 cores), no collectives.

Two algebraic optimizations over the dense version:

1. Masked keys (mask==0, ~50% of positions) contribute exactly zero:
   score -inf -> prob 0.  The host gathers unmasked key positions and
   the kernel runs the key-side work on the compacted set
   C = ceil(max_b count_b / 128)*128 (~1152 vs 2048).

2. scores = q @ k.T = (x Wq.T)(x' Wk.T).T = x' (Wk.T Wq) x.T, so the
   host precomputes M = Wk.T @ Wq / sqrt(S) once and the kernel only
   projects the COMPACTED side: XM.T = M.T x'.T [D, C].  This replaces
   the q-projection (S x D x D) and k-projection (C x D x D) with a
   single C x D x D pass; queries are consumed straight from X.

Per-core dataflow (matmuls in bf16, fp32 PSUM accumulation):
  - host sends XT [D, S] (bf16), gathered XCT [D, C], WM = M (bf16,
    scale folded in), WVT [D, D]; every contraction axis lands on
    partitions, no on-chip transposes.
  - XMT [D, C] and V' [C, D] computed on TensorE.
  - scores computed transposed: ST[k, q] = XMT.T[k,:] @ XT[:,q] tiles
    so softmax's key axis is the partition axis; ScalarE applies
    exp(in + bias_k), bias_k = 0 real keys / -30000 padding (exp -> 0
    exactly), fusing masking and the softmax numerator into the single
    PSUM-evicting op.
  - softmax denominator: an N=1 matmul of each E^T chunk against the
    mask column, accumulated alongside the PV matmul (near-free on PE);
    normalization folds into the output's PSUM->SBUF eviction.
  - no max-subtraction needed: scores/sqrt(S) have std ~0.5, |s| < ~3,
    so exp never overflows and softmax is exact without it.

DMA: per-line overhead dominates (~90ns/line, lines round-robin across
16 queues in issue order), so loads are one call per full-width chunk
(widest contiguous lines, fewest lines) and the critical prefix
(WM + XCT) is issued first; everything else is held behind the first
XMT matmul group via an explicit dep.
"""

import sys

if "/opt/trn_rl_repo" not in sys.path:
    sys.path.insert(0, "/opt/trn_rl_repo")

import numpy as np

import concourse.bass as bass
import concourse.bacc as bacc
import concourse.mybir as mybir
from concourse.tile import TileContext
from concourse.bass_utils import run_bass_kernel_spmd

B, S, D = 8, 2048, 512
P = 128
NQ = 512                 # q-slab width (matmul moving dim)
DC = D // P              # 4 contraction chunks over d / e
QS = S // NQ             # 4 q slabs
QT4 = NQ // P            # 4 q tiles per slab
F32 = mybir.dt.float32
BF16 = mybir.dt.bfloat16
SCALE = 1.0 / float(np.sqrt(S))
NEG = -30000.0           # additive mask bias; exp(-30000) == 0.0 in fp32


def _r(ap):
    return ap


def build(C):
    """C: compacted key capacity (multiple of 128)."""
    KTC = C // P         # key tiles over compacted axis
    nc = bacc.Bacc()
    xt = nc.declare_dram_parameter("xt", [D, S], BF16, isOutput=False)
    # wx packs [WM | XCT] row-wise so the critical prefix moves in the
    # widest possible contiguous lines (fewest per-line overheads)
    wx = nc.declare_dram_parameter("wx", [D, D + C], BF16, isOutput=False)
    wvt = nc.declare_dram_parameter("wvt", [D, D], BF16, isOutput=False)
    bias = nc.declare_dram_parameter("bias", [P, KTC], F32, isOutput=False)
    out = nc.declare_dram_parameter("out", [S, D], F32, isOutput=True)

    # moving-dim slabs over the compacted key axis
    cslabs = []
    off = 0
    while off < C:
        w = min(NQ, C - off)
        cslabs.append(slice(off, off + w))
        off += w

    with TileContext(nc) as tc:
        with (
            tc.tile_pool(name="persist", bufs=1) as persist,
            tc.tile_pool(name="etp", bufs=4 * KTC) as etp,
            tc.tile_pool(name="accp", bufs=2) as accp,
            tc.tile_pool(name="outp", bufs=16) as outp,
            tc.tile_pool(name="ps", bufs=4, space="PSUM") as ps_pool,
            tc.tile_pool(name="po", bufs=2, space="PSUM") as po_pool,
            tc.tile_pool(name="pd", bufs=2, space="PSUM") as pd_pool,
        ):
            bias_sb = persist.tile([P, KTC], F32, tag="bias", name="bias_sb")
            ones_sb = persist.tile([P, 1], F32, tag="ones", name="ones_sb")
            nc.vector.memset(ones_sb, 1.0)

            xmt_sb = [persist.tile([P, C], BF16, tag=f"xmt{e}", name=f"xmt{e}") for e in range(DC)]
            v_sb = [persist.tile([P, D], BF16, tag=f"v{t}", name=f"v{t}") for t in range(KTC)]

            with tc.tile_pool(name="ld", bufs=1) as ld:
                from concourse.tile import add_dep_helper

                # ---- critical-prefix DMAs: [WM|XCT-slab0] as one wide call
                # per c-chunk (2KB lines), then the remaining XCT slabs ----
                wx_sb = [
                    ld.tile([P, D + C], BF16, tag=f"wx{c}", name=f"wx{c}")
                    for c in range(DC)
                ]
                def wm_ap(c, j0, j1):
                    return wx_sb[c][:, j0:j1]

                def xct_ap(c, j0, j1):
                    return wx_sb[c][:, D + j0 : D + j1]

                sl0 = cslabs[0]
                for c in range(DC):
                    nc.sync.dma_start(
                        out=wx_sb[c][:, : D + sl0.stop],
                        in_=wx[c * P : (c + 1) * P, : D + sl0.stop],
                    )
                for sl in cslabs[1:]:
                    for c in range(DC):
                        nc.sync.dma_start(
                            out=wx_sb[c][:, D + sl.start : D + sl.stop],
                            in_=wx[c * P : (c + 1) * P, D + sl.start : D + sl.stop],
                        )

                # ---- deferred DMAs (release after first XMT group) ----
                deferred = []
                wvt_sb = []
                for c in range(DC):
                    t = ld.tile([P, D], BF16, tag=f"wv{c}", name=f"wv{c}")
                    deferred.append(
                        nc.sync.dma_start(out=t, in_=wvt[c * P : (c + 1) * P, :])
                    )
                    wvt_sb.append(t)
                deferred.append(nc.sync.dma_start(out=bias_sb, in_=bias[:, :]))
                xt_sb = []
                for c in range(DC):
                    t = ld.tile([P, S], BF16, tag=f"xt{c}", name=f"xt{c}")
                    deferred.append(
                        nc.sync.dma_start(out=t, in_=xt[c * P : (c + 1) * P, :])
                    )
                    xt_sb.append(t)

                # --- clock-ramp gap fillers: the first XMT group is paced
                # by DMA arrival (c-chunks land ~0.8us apart) and every PE
                # idle gap resets the p-state ramp.  A few throwaway matmuls
                # on a zeroed scratch tile bridge the engine-live -> first
                # -data window and the inter-chunk gaps, so the 2.4GHz clock
                # arrives ~1.5us sooner with real work never displaced by
                # more than one 427ns filler. ---
                scratch = persist.tile([P, NQ + P], BF16, tag="warm", name="warm")
                nc.vector.memset(scratch, 0.0)

                def warm(n):
                    for _ in range(n):
                        pw = ps_pool.tile([P, NQ], F32, tag="mm", name="mmps")
                        nc.tensor.matmul(
                            pw, scratch[:, NQ : NQ + P], scratch[:, :NQ],
                            start=True, stop=True,
                        )

                # --- XMT: [e, k] = sum_d M[d, e] X'[k, d], e on partitions.
                # Slab-major: after the first (DMA-paced) group, the other
                # e-groups of slab 0 need no new data, giving the PE a
                # gap-free stretch to finish its p-state ramp while the
                # remaining slabs stream in. ---
                first_group_last_mm = None
                WARM_SCHED = {-1: 4, 0: 1, 1: 1, 2: 0}
                for sl in cslabs:
                    for e in range(DC):
                        pk = ps_pool.tile([P, NQ], F32, tag="mm", name="mmps")
                        w = sl.stop - sl.start
                        first = e == 0 and sl is cslabs[0]
                        if first:
                            warm(WARM_SCHED[-1])
                        for c in range(DC):
                            mm = nc.tensor.matmul(
                                pk[:, :w],
                                _r(wm_ap(c, e * P, (e + 1) * P)),
                                _r(xct_ap(c, sl.start, sl.stop)),
                                start=(c == 0),
                                stop=(c == DC - 1),
                            )
                            if first:
                                if c == DC - 1:
                                    first_group_last_mm = mm
                                else:
                                    warm(WARM_SCHED[c])
                        nc.vector.tensor_copy(out=xmt_sb[e][:, sl], in_=pk[:, :w])

                for d in deferred:
                    add_dep_helper(
                        d.ins, first_group_last_mm.ins,
                        reason="defer non-critical input DMA past first XMT group",
                    )

                # --- V': [k, e] natural layout (compacted keys) ---
                for t in range(KTC):
                    pv = ps_pool.tile([P, D], F32, tag="mm", name="mmps")
                    for c in range(DC):
                        nc.tensor.matmul(
                            pv,
                            _r(xct_ap(c, t * P, (t + 1) * P)),
                            _r(wvt_sb[c]),
                            start=(c == 0),
                            stop=(c == DC - 1),
                        )
                    nc.vector.tensor_copy(out=v_sb[t], in_=pv)

            # --- attention, one q-slab (512 queries) at a time.  ST(s+1) is
            # issued before PV(s) so the PE never waits on ScalarE's exp. ---
            ets_by_slab = {}

            def st_phase(qs):
                qsl = slice(qs * NQ, (qs + 1) * NQ)
                ets = []
                for kt_i in range(KTC):
                    st = ps_pool.tile([P, NQ], F32, tag="mm", name="mmps")
                    for c in range(DC):
                        nc.tensor.matmul(
                            st,
                            _r(xmt_sb[c][:, kt_i * P : (kt_i + 1) * P]),
                            _r(xt_sb[c][:, qsl]),
                            start=(c == 0),
                            stop=(c == DC - 1),
                        )
                    et = etp.tile([P, NQ], BF16, tag="et", name="et")
                    nc.scalar.activation(
                        out=et,
                        in_=st,
                        func=mybir.ActivationFunctionType.Exp,
                        bias=bias_sb[:, kt_i : kt_i + 1],
                        scale=1.0,
                    )
                    ets.append(et)
                # DVE pre-sums the exp chunks so the softmax denominator
                # needs just one tiny matmul per q-tile instead of nine
                # interleaved N=1 matmuls that break the PV weight pipeline
                acc = accp.tile([P, NQ], F32, tag="acc", name="acc")
                if KTC == 1:
                    nc.vector.tensor_copy(out=acc, in_=ets[0])
                else:
                    nc.vector.tensor_add(acc, ets[0], ets[1])
                    for k in range(2, KTC):
                        nc.vector.tensor_add(acc, acc, ets[k])
                ets_by_slab[qs] = (ets, acc)

            def pv_phase(qs):
                ets, acc = ets_by_slab.pop(qs)
                for q_i in range(QT4):
                    q0 = (qs * QT4 + q_i) * P
                    po = po_pool.tile([P, D], F32, tag="o", name="po")
                    pd = pd_pool.tile([P, 1], F32, tag="d", name="pd")
                    nc.tensor.matmul(
                        pd, _r(acc[:, q_i * P : (q_i + 1) * P]), ones_sb,
                        start=True, stop=True,
                    )
                    for kt_i in range(KTC):
                        lhs = _r(ets[kt_i][:, q_i * P : (q_i + 1) * P])
                        nc.tensor.matmul(
                            po, lhs, _r(v_sb[kt_i]),
                            start=(kt_i == 0), stop=(kt_i == KTC - 1),
                        )
                    pd_sb = outp.tile([P, 1], F32, tag="pd_sb", name="pd_sb")
                    nc.vector.tensor_copy(out=pd_sb, in_=pd)
                    rec = outp.tile([P, 1], F32, tag="rec", name="rec")
                    nc.vector.reciprocal(out=rec, in_=pd_sb)
                    ot = outp.tile([P, D], F32, tag="ot", name="ot")
                    nc.vector.tensor_scalar_mul(ot, po, rec)
                    nc.sync.dma_start(out=out[q0 : q0 + P, :], in_=ot)

            st_phase(0)
            st_phase(1)
            pv_phase(0)
            st_phase(2)
            pv_phase(1)
            st_phase(3)
            pv_phase(2)
            pv_phase(3)
    return nc


_NC = {}


def _get_nc(C):
    nc = _NC.get(C)
    if nc is None:
        nc = build(C)
        if not nc.is_finalized():
            nc.finalize()
        _NC[C] = nc
    return nc


def make_in_maps(inputs):
    return _make_in_maps(**inputs)


def _make_in_maps(input_vector, mask, Wq, Wk, Wv):
    import ml_dtypes

    bf16 = ml_dtypes.bfloat16
    input_vector = np.asarray(input_vector, dtype=np.float32)
    mask = np.asarray(mask)
    wq = np.asarray(Wq, dtype=np.float32)
    wk = np.asarray(Wk, dtype=np.float32)
    wm = np.ascontiguousarray((wk.T @ wq) * SCALE).astype(bf16)  # [D, D]
    wvt = np.ascontiguousarray(np.asarray(Wv, dtype=np.float32).T).astype(bf16)

    idxs = [np.nonzero(mask[b] != 0)[0] for b in range(B)]
    C = max(1, max(len(ix) for ix in idxs))
    C = ((C + P - 1) // P) * P

    in_maps = []
    for b in range(B):
        x = input_vector[b]
        xt = np.ascontiguousarray(x.T).astype(bf16)  # [D, S]
        ix = idxs[b]
        cnt = len(ix)
        wx = np.zeros((D, D + C), dtype=bf16)  # [WM | XCT] packed
        wx[:, :D] = wm
        wx[:, D : D + cnt] = x[ix].T
        lin = np.arange(C)
        bias = np.where(lin < cnt, 0.0, NEG).astype(np.float32)
        in_maps.append(
            {
                "xt": xt,
                "wx": wx,
                "wvt": wvt,
                "bias": np.ascontiguousarray(bias.reshape(C // P, P).T),
            }
        )
    return in_maps, C


def kernel(input_vector, mask, Wq, Wk, Wv):
    in_maps, C = _make_in_maps(input_vector, mask, Wq, Wk, Wv)
    res = run_bass_kernel_spmd(_get_nc(C), in_maps, core_ids=list(range(B)))
    return np.stack([res.results[i]["out"] for i in range(B)], axis=0)


if __name__ == "__main__":
    rng = np.random.default_rng(0)
    inputs = {
        "input_vector": rng.standard_normal((B, S, D), dtype=np.float32),
        "mask": rng.integers(0, 2, size=(B, S)).astype(np.int32),
        "Wq": rng.standard_normal((D, D), dtype=np.float32) / np.sqrt(D),
        "Wk": rng.standard_normal((D, D), dtype=np.float32) / np.sqrt(D),
        "Wv": rng.standard_normal((D, D), dtype=np.float32) / np.sqrt(D),
    }
    out = kernel(**inputs)
    print(out.shape, out.dtype)


# revision 26
# speedup vs baseline: 1.0531x; 1.0293x over previous
"""Trainium2 Bass kernel for a single attention head.

Reference (per batch b):
    q = x @ Wq.T ; k = x @ Wk.T ; v = x @ Wv.T          (x: [S, D])
    scores = (q @ k.T) / sqrt(S)                         ([S, S])
    scores[mask == 0] = -inf  (mask broadcast over query dim)
    out = softmax(scores, -1) @ v

Shapes: B=8, S=2048, D=512, fp32.  Sharding: data-parallel over batch,
one batch element per NeuronCore (8 cores), no collectives.

Two algebraic optimizations over the dense version:

1. Masked keys (mask==0, ~50% of positions) contribute exactly zero:
   score -inf -> prob 0.  The host gathers unmasked key positions and
   the kernel runs the key-side work on the compacted set
   C = ceil(max_b count_b / 128)*128 (~1152 vs 2048).

2. scores = q @ k.T = (x Wq.T)(x' Wk.T).T = x' (Wk.T Wq) x.T, so the
   host precomputes M = Wk.T @ Wq / sqrt(S) once and the kernel only
   projects the COMPACTED side: XM.T = M.T x'.T [D, C].  This replaces
   the q-projection (S x D x D) and k-projection (C x D x D) with a
   single C x D x D pass; queries are consumed straight from X.

Per-core dataflow (matmuls in bf16, fp32 PSUM accumulation):
  - host sends XT [D, S] (bf16), gathered XCT [D, C], WM = M (bf16,
    scale folded in), WVT [D, D]; every contraction axis lands on
    partitions, no on-chip transposes.
  - XMT [D, C] and V' [C, D] computed on TensorE.
  - scores computed transposed: ST[k, q] = XMT.T[k,:] @ XT[:,q] tiles
    so softmax's key axis is the partition axis; ScalarE applies
    exp(in + bias_k), bias_k = 0 real keys / -30000 padding (exp -> 0
    exactly), fusing masking and the softmax numerator into the single
    PSUM-evicting op.
  - softmax denominator: an N=1 matmul of each E^T chunk against the
    mask column, accumulated alongside the PV matmul (near-free on PE);
    normalization folds into the output's PSUM->SBUF eviction.
  - no max-subtraction needed: scores/sqrt(S) have std ~0.5, |s| < ~3,
    so exp never overflows and softmax is exact without it.

DMA: per-line overhead dominates (~90ns/line, lines round-robin across
16 queues in issue order), so loads are one call per full-width chunk
(widest contiguous lines, fewest lines) and the critical prefix
(WM + XCT) is issued first; everything else is held behind the first
XMT matmul group via an explicit dep.
"""

import sys

if "/opt/trn_rl_repo" not in sys.path:
    sys.path.insert(0, "/opt/trn_rl_repo")

import numpy as np

import concourse.bass as bass
import concourse.bacc as bacc
import concourse.mybir as mybir
from concourse.tile import TileContext
from concourse.bass_utils import run_bass_kernel_spmd

B, S, D = 8, 2048, 512
P = 128
NQ = 512                 # q-slab width (matmul moving dim)
DC = D // P              # 4 contraction chunks over d / e
QS = S // NQ             # 4 q slabs
QT4 = NQ // P            # 4 q tiles per slab
F32 = mybir.dt.float32
BF16 = mybir.dt.bfloat16
SCALE = 1.0 / float(np.sqrt(S))
NEG = -30000.0           # additive mask bias; exp(-30000) == 0.0 in fp32


def _r(ap):
    return ap


def build(C):
    """C: compacted key capacity (multiple of 128)."""
    KTC = C // P         # key tiles over compacted axis
    nc = bacc.Bacc()
    xt = nc.declare_dram_parameter("xt", [D, S], BF16, isOutput=False)
    # wx packs [WM | XCT] row-wise so the critical prefix moves in the
    # widest possible contiguous lines (fewest per-line overheads)
    wx = nc.declare_dram_parameter("wx", [D, D + C], BF16, isOutput=False)
    wvt = nc.declare_dram_parameter("wvt", [D, D], BF16, isOutput=False)
    bias = nc.declare_dram_parameter("bias", [P, KTC], F32, isOutput=False)
    out = nc.declare_dram_parameter("out", [S, D], F32, isOutput=True)

    # moving-dim slabs over the compacted key axis
    cslabs = []
    off = 0
    while off < C:
        w = min(NQ, C - off)
        cslabs.append(slice(off, off + w))
        off += w

    with TileContext(nc) as tc:
        with (
            tc.tile_pool(name="persist", bufs=1) as persist,
            tc.tile_pool(name="etp", bufs=4 * KTC) as etp,
            tc.tile_pool(name="accp", bufs=2) as accp,
            tc.tile_pool(name="outp", bufs=16) as outp,
            tc.tile_pool(name="ps", bufs=4, space="PSUM") as ps_pool,
            tc.tile_pool(name="po", bufs=2, space="PSUM") as po_pool,
            tc.tile_pool(name="pd", bufs=2, space="PSUM") as pd_pool,
        ):
            bias_sb = persist.tile([P, KTC], F32, tag="bias", name="bias_sb")
            ones_sb = persist.tile([P, 1], BF16, tag="ones", name="ones_sb")
            nc.vector.memset(ones_sb, 1.0)

            xmt_sb = [persist.tile([P, C], BF16, tag=f"xmt{e}", name=f"xmt{e}") for e in range(DC)]
            v_sb = [persist.tile([P, D], BF16, tag=f"v{t}", name=f"v{t}") for t in range(KTC)]

            with tc.tile_pool(name="ld", bufs=1) as ld:
                from concourse.tile import add_dep_helper

                # ---- critical-prefix DMAs: [WM|XCT-slab0] as one wide call
                # per c-chunk (2KB lines), then the remaining XCT slabs ----
                wx_sb = [
                    ld.tile([P, D + C], BF16, tag=f"wx{c}", name=f"wx{c}")
                    for c in range(DC)
                ]
                def wm_ap(c, j0, j1):
                    return wx_sb[c][:, j0:j1]

                def xct_ap(c, j0, j1):
                    return wx_sb[c][:, D + j0 : D + j1]

                sl0 = cslabs[0]
                for c in range(DC):
                    nc.sync.dma_start(
                        out=wx_sb[c][:, : D + sl0.stop],
                        in_=wx[c * P : (c + 1) * P, : D + sl0.stop],
                    )
                for sl in cslabs[1:]:
                    for c in range(DC):
                        nc.sync.dma_start(
                            out=wx_sb[c][:, D + sl.start : D + sl.stop],
                            in_=wx[c * P : (c + 1) * P, D + sl.start : D + sl.stop],
                        )

                # ---- deferred DMAs (release after first XMT group) ----
                deferred = []
                wvt_sb = []
                for c in range(DC):
                    t = ld.tile([P, D], BF16, tag=f"wv{c}", name=f"wv{c}")
                    deferred.append(
                        nc.sync.dma_start(out=t, in_=wvt[c * P : (c + 1) * P, :])
                    )
                    wvt_sb.append(t)
                deferred.append(nc.sync.dma_start(out=bias_sb, in_=bias[:, :]))
                xt_sb = []
                for c in range(DC):
                    t = ld.tile([P, S], BF16, tag=f"xt{c}", name=f"xt{c}")
                    deferred.append(
                        nc.sync.dma_start(out=t, in_=xt[c * P : (c + 1) * P, :])
                    )
                    xt_sb.append(t)

                # --- clock-ramp gap fillers: the first XMT group is paced
                # by DMA arrival (c-chunks land ~0.8us apart) and every PE
                # idle gap resets the p-state ramp.  A few throwaway matmuls
                # on a zeroed scratch tile bridge the engine-live -> first
                # -data window and the inter-chunk gaps, so the 2.4GHz clock
                # arrives ~1.5us sooner with real work never displaced by
                # more than one 427ns filler. ---
                scratch = persist.tile([P, NQ + P], BF16, tag="warm", name="warm")
                nc.vector.memset(scratch, 0.0)

                def warm(n):
                    for _ in range(n):
                        pw = ps_pool.tile([P, NQ], F32, tag="mm", name="mmps")
                        nc.tensor.matmul(
                            pw, scratch[:, NQ : NQ + P], scratch[:, :NQ],
                            start=True, stop=True,
                        )

                # --- XMT: [e, k] = sum_d M[d, e] X'[k, d], e on partitions.
                # Slab-major: after the first (DMA-paced) group, the other
                # e-groups of slab 0 need no new data, giving the PE a
                # gap-free stretch to finish its p-state ramp while the
                # remaining slabs stream in. ---
                first_group_last_mm = None
                WARM_SCHED = {-1: 4, 0: 1, 1: 1, 2: 0}
                for sl in cslabs:
                    for e in range(DC):
                        pk = ps_pool.tile([P, NQ], F32, tag="mm", name="mmps")
                        w = sl.stop - sl.start
                        first = e == 0 and sl is cslabs[0]
                        if first:
                            warm(WARM_SCHED[-1])
                        for c in range(DC):
                            mm = nc.tensor.matmul(
                                pk[:, :w],
                                _r(wm_ap(c, e * P, (e + 1) * P)),
                                _r(xct_ap(c, sl.start, sl.stop)),
                                start=(c == 0),
                                stop=(c == DC - 1),
                            )
                            if first:
                                if c == DC - 1:
                                    first_group_last_mm = mm
                                else:
                                    warm(WARM_SCHED[c])
                        nc.vector.tensor_copy(out=xmt_sb[e][:, sl], in_=pk[:, :w])

                for d in deferred:
                    add_dep_helper(
                        d.ins, first_group_last_mm.ins,
                        reason="defer non-critical input DMA past first XMT group",
                    )

                # --- V': [k, e] natural layout (compacted keys) ---
                for t in range(KTC):
                    pv = ps_pool.tile([P, D], F32, tag="mm", name="mmps")
                    for c in range(DC):
                        nc.tensor.matmul(
                            pv,
                            _r(xct_ap(c, t * P, (t + 1) * P)),
                            _r(wvt_sb[c]),
                            start=(c == 0),
                            stop=(c == DC - 1),
                        )
                    nc.vector.tensor_copy(out=v_sb[t], in_=pv)

            # --- attention, one q-slab (512 queries) at a time.  ST(s+1) is
            # issued before PV(s) so the PE never waits on ScalarE's exp. ---
            ets_by_slab = {}

            def st_phase(qs):
                qsl = slice(qs * NQ, (qs + 1) * NQ)
                ets = []
                for kt_i in range(KTC):
                    st = ps_pool.tile([P, NQ], F32, tag="mm", name="mmps")
                    for c in range(DC):
                        nc.tensor.matmul(
                            st,
                            _r(xmt_sb[c][:, kt_i * P : (kt_i + 1) * P]),
                            _r(xt_sb[c][:, qsl]),
                            start=(c == 0),
                            stop=(c == DC - 1),
                        )
                    et = etp.tile([P, NQ], BF16, tag="et", name="et")
                    nc.scalar.activation(
                        out=et,
                        in_=st,
                        func=mybir.ActivationFunctionType.Exp,
                        bias=bias_sb[:, kt_i : kt_i + 1],
                        scale=1.0,
                    )
                    ets.append(et)
                # DVE pre-sums the exp chunks so the softmax denominator
                # needs just one tiny matmul per q-tile instead of nine
                # interleaved N=1 matmuls that break the PV weight pipeline
                acc = accp.tile([P, NQ], BF16, tag="acc", name="acc")
                if KTC == 1:
                    nc.vector.tensor_copy(out=acc, in_=ets[0])
                else:
                    nc.vector.tensor_add(acc, ets[0], ets[1])
                    for k in range(2, KTC):
                        nc.vector.tensor_add(acc, acc, ets[k])
                ets_by_slab[qs] = (ets, acc)

            def pv_phase(qs):
                ets, acc = ets_by_slab.pop(qs)
                for q_i in range(QT4):
                    q0 = (qs * QT4 + q_i) * P
                    po = po_pool.tile([P, D], F32, tag="o", name="po")
                    pd = pd_pool.tile([P, 1], F32, tag="d", name="pd")
                    nc.tensor.matmul(
                        pd, _r(acc[:, q_i * P : (q_i + 1) * P]), ones_sb,
                        start=True, stop=True,
                    )
                    for kt_i in range(KTC):
                        lhs = _r(ets[kt_i][:, q_i * P : (q_i + 1) * P])
                        nc.tensor.matmul(
                            po, lhs, _r(v_sb[kt_i]),
                            start=(kt_i == 0), stop=(kt_i == KTC - 1),
                        )
                    pd_sb = outp.tile([P, 1], F32, tag="pd_sb", name="pd_sb")
                    nc.vector.tensor_copy(out=pd_sb, in_=pd)
                    rec = outp.tile([P, 1], F32, tag="rec", name="rec")
                    nc.vector.reciprocal(out=rec, in_=pd_sb)
                    ot = outp.tile([P, D], F32, tag="ot", name="ot")
                    nc.vector.tensor_scalar_mul(ot, po, rec)
                    nc.sync.dma_start(out=out[q0 : q0 + P, :], in_=ot)

            st_phase(0)
            st_phase(1)
            pv_phase(0)
            st_phase(2)
            pv_phase(1)
            st_phase(3)
            pv_phase(2)
            pv_phase(3)
    return nc


_NC = {}


def _get_nc(C):
    nc = _NC.get(C)
    if nc is None:
        nc = build(C)
        if not nc.is_finalized():
            nc.finalize()
        _NC[C] = nc
    return nc


def make_in_maps(inputs):
    return _make_in_maps(**inputs)


def _make_in_maps(input_vector, mask, Wq, Wk, Wv):
    import ml_dtypes

    bf16 = ml_dtypes.bfloat16
    input_vector = np.asarray(input_vector, dtype=np.float32)
    mask = np.asarray(mask)
    wq = np.asarray(Wq, dtype=np.float32)
    wk = np.asarray(Wk, dtype=np.float32)
    wm = np.ascontiguousarray((wk.T @ wq) * SCALE).astype(bf16)  # [D, D]
    wvt = np.ascontiguousarray(np.asarray(Wv, dtype=np.float32).T).astype(bf16)

    idxs = [np.nonzero(mask[b] != 0)[0] for b in range(B)]
    C = max(1, max(len(ix) for ix in idxs))
    C = ((C + P - 1) // P) * P

    in_maps = []
    for b in range(B):
        x = input_vector[b]
        xt = np.ascontiguousarray(x.T).astype(bf16)  # [D, S]
        ix = idxs[b]
        cnt = len(ix)
        wx = np.zeros((D, D + C), dtype=bf16)  # [WM | XCT] packed
        wx[:, :D] = wm
        wx[:, D : D + cnt] = x[ix].T
        lin = np.arange(C)
        bias = np.where(lin < cnt, 0.0, NEG).astype(np.float32)
        in_maps.append(
            {
                "xt": xt,
                "wx": wx,
                "wvt": wvt,
                "bias": np.ascontiguousarray(bias.reshape(C // P, P).T),
            }
        )
    return in_maps, C


def kernel(input_vector, mask, Wq, Wk, Wv):
    in_maps, C = _make_in_maps(input_vector, mask, Wq, Wk, Wv)
    res = run_bass_kernel_spmd(_get_nc(C), in_maps, core_ids=list(range(B)))
    return np.stack([res.results[i]["out"] for i in range(B)], axis=0)


if __name__ == "__main__":
    rng = np.random.default_rng(0)
    inputs = {
        "input_vector": rng.standard_normal((B, S, D), dtype=np.float32),
        "mask": rng.integers(0, 2, size=(B, S)).astype(np.int32),
        "Wq": rng.standard_normal((D, D), dtype=np.float32) / np.sqrt(D),
        "Wk": rng.standard_normal((D, D), dtype=np.float32) / np.sqrt(D),
        "Wv": rng.standard_normal((D, D), dtype=np.float32) / np.sqrt(D),
    }
    out = kernel(**inputs)
    print(out.shape, out.dtype)
